# revision 44
# baseline (speedup 1.0000x reference)
"""ClusterISAAttention Trainium2 kernel (8 NeuronCores, SPMD).

Host: per batch (2), stable-sort queries by window id, split into 4
contiguous quarters of 2048 queries -> 8 (batch, quarter) shards, one
per core.  Queries are grouped into "slots": one window per slot, <=32
queries per slot; slot count padded to NSLOT (multiple of 32, chosen
from the actual inputs, typically 96) so the SPMD program is static.
Padded queries/slots are dummies, discarded on unpack.

Bias folding (host): bk is dropped entirely (adds a per-query constant
to every logit in a row - softmax invariant); bv is folded into bo
(softmax rows sum to 1, so attn@(V+bv) = attn@V + bv and
bo' = bo + Wo @ bv); SCALE is folded into Wq/bq.  In-kernel k/v
evictions are therefore pure casts, distributed over the scalar,
vector and gpsimd engines by a greedy load balancer.

Device (per core), bf16 matmuls with fp32 psum:
  qT   = (Wq' @ xqT + bq') -> [256, NQ] dims-major
  kT   -> evicted into block-diag per-slot tables kbd[g][128, slot, 256]
          (4 heads per group; block-diag so one matmul covers 4 heads)
  vT   -> [128, slot, 128] tables (key half at 64*(h%2), rest zero),
          xbar-transposed per head-pair into block-diag
          vbd[pair][128(2x64 keys), slot, 64(2x32 dims)] - the zero half
          of vT lands as vbd's off-diagonal zeros
  S    = qT_slot.T @ kbd -> psum [32q x 4slots, 512] (query-major)
  A    = exp(S) (logits are O(1): no max subtraction), normalized by
         1/rowsum on DVE, key pads zeroed
  AT   = xbar transpose of A -> atbd[128(2x64 keys), pair, tile, 32q]
  ctxT = vbd.T @ atbd -> psum [128, 64] per slot -> ctxT2 bf16 [128, 2, NQ]
  outT = Wo @ ctxT + bo' -> fp32 [256, NQ] -> DRAM
Host: outT columns scattered back to original query order.

All xbar transposes go through nc.sync ONLY (concurrent transposes on
two HWDGE rings corrupt each other - HW-verified), and the xbar maps
src element index i -> dst partition i%128, dst mid i//128 (verified).
"""

import os
import sys
import numpy as np
import ml_dtypes

for _p in ("/opt/trn_rl_repo", "/root/.axon_site/_ro/trn_rl_repo"):
    if os.path.isdir(_p) and _p not in sys.path:
        sys.path.append(_p)

import concourse.bass as bass
import concourse.tile as tile
from concourse import bacc
from concourse import mybir

F32 = mybir.dt.float32
BF16 = mybir.dt.bfloat16
AF = mybir.ActivationFunctionType
ALU = mybir.AluOpType
AX = mybir.AxisListType

B, N, C, H, HD, W, K = 2, 8192, 256, 8, 32, 361, 49
SCALE = float(HD) ** -0.5

NCORES = 8
QTRS = 4
NLOC = N // QTRS         # 2048 queries per core
CAP = 32                 # queries per slot
SPP = 16                 # slots per phase (phase count = NSLOT/SPP)
KP = 64                  # padded keys per head block


class _EvictionBalancer:
    """Greedy distribution of psum-eviction ops over scalar/vector/gpsimd.

    Cost model (ns): fixed per-instruction overhead + free-elems/freq.
    """

    def __init__(self, nc):
        self.nc = nc
        self.load = {"scalar": 0.0, "vector": 0.0, "gpsimd": 0.0}

    def _cost(self, eng, elems):
        # calibrated against HW traces: vector CAST (bf16 out) ~399ns for
        # 294 elems, scalar ACTIVATE ~487, gpsimd TT ~1040 for 512
        if eng == "scalar":
            return 220.0 + elems / 1.2
        if eng == "vector":
            return 170.0 + elems / 1.05
        return 300.0 + elems / 0.75  # gpsimd (Q7 launch, slow impl)

    def charge(self, eng, elems):
        self.load[eng] += self._cost(eng, elems)

    def pick(self, elems, allowed=("scalar", "vector", "gpsimd")):
        eng = min(allowed, key=lambda e: self.load[e] + self._cost(e, elems))
        self.charge(eng, elems)
        return eng

    def copy(self, out, in_, allowed=("scalar", "vector")):
        # NOTE: gpsimd cannot access PSUM; psum evictions are scalar/vector
        eng = self.pick(out.free_size(), allowed)
        if eng == "scalar":
            self.nc.scalar.copy(out, in_)
        elif eng == "vector":
            self.nc.vector.tensor_copy(out, in_)
        else:
            self.nc.gpsimd.tensor_copy(out, in_)

    def bias_add(self, out, in_, bias_col, b_sb_scalar,
                 allowed=("scalar", "vector")):
        """out = in_ + bias (bias per-partition).  scalar uses activation,
        vector/gpsimd use tensor_tensor with a broadcast bias."""
        eng = self.pick(out.free_size(), allowed)
        if eng == "scalar":
            self.nc.scalar.activation(out, in_, AF.Identity, bias=b_sb_scalar)
        else:
            e = self.nc.vector if eng == "vector" else self.nc.gpsimd
            e.tensor_tensor(out=out, in0=in_, in1=bias_col, op=ALU.add)


def _build_program(nslot):
    assert nslot % SPP == 0 and nslot % 32 == 0 and nslot >= 32
    nq = nslot * CAP
    nc = bacc.Bacc("TRN2", target_bir_lowering=False, debug=False,
                   num_devices=NCORES)

    xqT = nc.declare_dram_parameter("xqT", [C, nq], BF16, isOutput=False).ap()
    xpT = nc.declare_dram_parameter("xpT", [C, nslot * K], BF16, isOutput=False).ap()
    wts = {
        nm: nc.declare_dram_parameter(f"w{nm}T", [C, C], BF16, isOutput=False).ap()
        for nm in ("q", "k", "v", "o")
    }
    bss = {
        nm: nc.declare_dram_parameter(f"b{nm}", [C], F32, isOutput=False).ap()
        for nm in ("q", "o")
    }
    outT = nc.declare_dram_parameter("outT", [C, nq], F32, isOutput=True).ap()

    with tile.TileContext(nc) as tc:
        _kernel_body(tc, xqT, xpT, wts, bss, outT, nslot)
    nc.compile()
    return nc


def _kernel_body(tc, xqT, xpT, wts, bss, outT, NSLOT):
    from contextlib import ExitStack

    PH = NSLOT // SPP        # phases (double-buffered)
    TPP = SPP // 4           # 4-slot tiles per phase
    SPC = SPP // 2           # slots per kv-proj chunk
    NQ = NSLOT * CAP

    nc = tc.nc
    bal = _EvictionBalancer(nc)
    ctx = ExitStack()
    with ctx:
        singles = ctx.enter_context(tc.tile_pool(name="singles", bufs=1))
        phpool = ctx.enter_context(tc.tile_pool(name="phase", bufs=2))
        pp = ctx.enter_context(tc.tile_pool(name="proj_ps", bufs=2, space="PSUM"))
        sp = ctx.enter_context(tc.tile_pool(name="s_ps", bufs=2, space="PSUM"))
        cp = ctx.enter_context(tc.tile_pool(name="ctx_ps", bufs=4, space="PSUM"))
        ostage = ctx.enter_context(tc.tile_pool(name="ostage", bufs=2))

        # ---- persistent SBUF ----
        w_sb = {nm: singles.tile([128, 2, C], BF16, tag=f"w_{nm}", name=f"w_{nm}") for nm in wts}
        b_sb = {nm: singles.tile([128, 2], F32, tag=f"b_{nm}", name=f"b_{nm}") for nm in bss}
        xq_sb = [singles.tile([128, NQ], BF16, tag=f"xq{c}", name=f"xq{c}") for c in range(2)]
        xp_sb = [singles.tile([128, NSLOT * K], BF16, tag=f"xp{c}", name=f"xp{c}") for c in range(2)]
        qT_sb = [singles.tile([128, NQ], BF16, tag=f"qT{g}", name=f"qT{g}") for g in range(2)]
        ctxT_sb = singles.tile([128, 2, NQ], BF16, tag="ctxT", name="ctxT")

        # ---- phase tiles: both parities created up front so their
        # one-time gap memsets can run at t=0 on idle engines (inside the
        # loop they'd queue behind load-blocked evictions) ----
        ptiles = []
        for par in range(2):
            ptiles.append(dict(
                kbd=[phpool.tile([128, SPP, 4 * KP], BF16, tag=f"kbd{g}", name=f"kbd{g}") for g in range(2)],
                vT=[phpool.tile([128, SPP, 2 * KP], BF16, tag=f"vT{m}", name=f"vT{m}") for m in range(2)],
                vbdf=[phpool.tile([128, SPP, 128], BF16, tag=f"vbdf{m}", name=f"vbdf{m}") for m in range(2)],
                a=phpool.tile([128, 4, TPP, 2 * KP], BF16, tag="a_sb", name="a_sb"),
                atbd=phpool.tile([128, 4, TPP, 4, CAP], BF16, tag="atbd", name="atbd"),
                den=phpool.tile([128, TPP, 8], F32, tag="den", name="den"),
                rec=phpool.tile([128, TPP, 8], F32, tag="rec", name="rec"),
            ))
        for par in range(2):
            T = ptiles[par]
            nc.vector.memset(T["kbd"][0][:], 0.0)
            nc.gpsimd.memset(T["kbd"][1][:], 0.0)
            nc.vector.memset(T["vT"][0][:], 0.0)
            nc.gpsimd.memset(T["vT"][1][:], 0.0)
            for a in range(2):
                (nc.vector if a == 0 else nc.gpsimd).memset(
                    T["a"][:, :, :, KP * a + K:KP * a + KP], 0.0)

        # ---- load inputs.  q-proj dependencies ride the otherwise-idle
        # sync (HWDGE) queue - its first transpose comes much later, so no
        # concurrency with transposes on the ring.  kv-side inputs go to
        # the gpsimd SWDGE queue, xp in per-phase chunks. ----
        nc.sync.dma_start(
            out=w_sb["q"][:], in_=wts["q"].rearrange("(s p) m -> p s m", p=128))
        nc.sync.dma_start(
            out=b_sb["q"][:], in_=bss["q"].rearrange("(s p) -> p s", p=128))
        # small first chunks so the first q-proj matmuls start ASAP
        xq_edges = [0, 512, 1024, NQ]
        for ck in range(len(xq_edges) - 1):
            lo, hi = xq_edges[ck], xq_edges[ck + 1]
            for c in range(2):
                nc.sync.dma_start(
                    out=xq_sb[c][:, lo:hi],
                    in_=xqT[c * 128:(c + 1) * 128, lo:hi])
        for nm in ("k", "v"):
            nc.gpsimd.dma_start(
                out=w_sb[nm][:], in_=wts[nm].rearrange("(s p) m -> p s m", p=128))
        for ph in range(PH):
            lo, hi = ph * SPP * K, (ph + 1) * SPP * K
            for c in range(2):
                nc.gpsimd.dma_start(out=xp_sb[c][:, lo:hi],
                                    in_=xpT[c * 128:(c + 1) * 128, lo:hi])
            if ph == 0:
                nc.gpsimd.dma_start(
                    out=w_sb["o"][:], in_=wts["o"].rearrange("(s p) m -> p s m", p=128))
                nc.gpsimd.dma_start(
                    out=b_sb["o"][:], in_=bss["o"].rearrange("(s p) -> p s", p=128))

        # ---- q projection: chunk nch covers exactly phase nch's queries
        # (SPP*CAP = 512).  Chunks 0-1 run up front; chunk p+2 is deferred
        # into phase p's body to shorten the serial head. ----
        def do_qproj(nch):
            for m in range(2):
                ps = pp.tile([128, 512], F32, tag="ps", name="ps")
                for c in range(2):
                    nc.tensor.matmul(
                        ps[:], w_sb["q"][:, c, m * 128:(m + 1) * 128],
                        xq_sb[c][:, nch * 512:(nch + 1) * 512],
                        start=(c == 0), stop=(c == 1))
                bal.bias_add(
                    qT_sb[m][:, nch * 512:(nch + 1) * 512], ps[:],
                    b_sb["q"][:, m:m + 1].broadcast_to([128, 512]),
                    b_sb["q"][:, m:m + 1])

        for nch in range(2):
            do_qproj(nch)

        for ph in range(PH):
            if ph + 2 < NQ // 512:
                do_qproj(ph + 2)
            T = ptiles[ph % 2]
            kbd_sb, vT_sb, vbdf_sb = T["kbd"], T["vT"], T["vbdf"]
            a_sb, atbd_sb, den_sb, rec_sb = T["a"], T["atbd"], T["den"], T["rec"]

            # k/v projections for this phase's SPP slots (evictions are
            # pure casts - biases folded on host - balanced over engines)
            for proj in ("k", "v"):
                for ch in range(2):
                    col0 = ph * SPP * K + ch * SPC * K
                    for m in range(2):
                        ps = pp.tile([128, 512], F32, tag="ps", name="ps")
                        for c in range(2):
                            nc.tensor.matmul(
                                ps[:, 0:SPC * K], w_sb[proj][:, c, m * 128:(m + 1) * 128],
                                xp_sb[c][:, col0:col0 + SPC * K],
                                start=(c == 0), stop=(c == 1))
                        if proj == "k":
                            for bb in range(4):
                                bal.copy(
                                    kbd_sb[m][32 * bb:32 * bb + 32,
                                              ch * SPC:(ch + 1) * SPC,
                                              KP * bb:KP * bb + K],
                                    ps[32 * bb:32 * bb + 32, 0:SPC * K].rearrange(
                                        "p (s k) -> p s k", k=K))
                        else:
                            for hh in range(4):
                                ko = KP * (hh % 2)
                                bal.copy(
                                    vT_sb[m][32 * hh:32 * hh + 32,
                                             ch * SPC:(ch + 1) * SPC, ko:ko + K],
                                    ps[32 * hh:32 * hh + 32, 0:SPC * K].rearrange(
                                        "p (s k) -> p s k", k=K))

            # vT -> block-diag vbd: fused xbar transpose per m-half
            # (dst [128 keyslots, s, 128 = src partitions]; ctx pair views
            # slice the inner dim)
            for m in range(2):
                nc.sync.dma_start_transpose(
                    out=vbdf_sb[m][:], in_=vT_sb[m][:])

            # logits + exp + rowsum per 4-slot tile
            for t in range(TPP):
                st = sp.tile([128, 512], F32, tag="st", name="st")
                for sl in range(4):
                    s_ph = t * 4 + sl
                    qcol = (ph * SPP + s_ph) * CAP
                    for g in range(2):
                        nc.tensor.matmul(
                            st[32 * sl:32 * sl + 32, 256 * g:256 * g + 256],
                            qT_sb[g][:, qcol:qcol + CAP],
                            kbd_sb[g][:, s_ph, :],
                            start=(g == 0), stop=(g == 1),
                            skip_group_check=True, tile_position=(0, 32 * sl))
                # exp / rowsum / normalize only touch the 49 valid key
                # columns; the 64-pads are zeroed once below and stay zero
                nc.scalar.activation(
                    a_sb[:, :, t, :].rearrange("p x (a j) -> p x a j", a=2)[:, :, :, 0:K],
                    st[:].rearrange("p (x a j) -> p x a j", x=4, a=2)[:, :, :, 0:K],
                    AF.Exp)
                bal.charge("scalar", 560)
                bal.charge("vector", 550)
                nc.vector.tensor_reduce(
                    out=den_sb[:, t, :].rearrange("p (x a) -> p x a", x=4),
                    in_=a_sb[:, :, t, :].rearrange("p x (a j) -> p x a j", a=2)[:, :, :, 0:K],
                    axis=AX.X, op=ALU.add)
                # per-tile reciprocal: shortens the reduce->normalize chain
                nc.vector.reciprocal(rec_sb[:, t, :], den_sb[:, t, :])
                bal.charge("vector", 200)
                a4 = a_sb[:, :, t, :].rearrange("p x (a j) -> p x a j", a=2)[:, :, :, 0:K]
                r4 = rec_sb[:, t, :].rearrange("p (x a) -> p x a", x=4) \
                    .unsqueeze(3).broadcast_to([128, 4, 2, K])
                eng = bal.pick(392, ("vector", "gpsimd"))
                e = nc.vector if eng == "vector" else nc.gpsimd
                e.tensor_tensor(out=a4, in0=a4, in1=r4, op=ALU.mult)
            # A -> atbd: one fused xbar transpose for the whole phase
            # (dst inner = src partitions = (cq, q))
            nc.sync.dma_start_transpose(
                out=atbd_sb[:].rearrange("p x t c q -> p (x t) (c q)"),
                in_=a_sb[:])

            # ctx: 4 pair-matmuls per slot; 2 slots share one psum tile so
            # the ctxT eviction runs half as often at double width
            for s2 in range(SPP // 2):
                cps = cp.tile([128, 2, 2 * CAP], F32, tag="cps", name="cps")
                for si in range(2):
                    s_ph = 2 * s2 + si
                    t, cq = divmod(s_ph, 4)
                    for p in range(4):
                        m, q = divmod(p, 2)
                        nc.tensor.matmul(
                            cps[64 * (p % 2):64 * (p % 2) + 64, si,
                                CAP * (p // 2):CAP * (p // 2) + CAP],
                            vbdf_sb[m][:, s_ph, 64 * q:64 * q + 64],
                            atbd_sb[:, p, t, cq, :],
                            start=(p < 2), stop=(p >= 2),
                            skip_group_check=True, tile_position=(0, 64 * (p % 2)))
                col = (ph * SPP + 2 * s2) * CAP
                cview = cps[:].rearrange("p s (j q) -> p j s q", j=2)
                bal.copy(
                    ctxT_sb[:, :, col:col + 2 * CAP].rearrange(
                        "p j (s q) -> p j s q", s=2), cview)

            # ---- output projection for this phase's columns (psum reuses
            # the S-tile pool slots: no extra banks) ----
            PCOL = SPP * CAP
            col0 = ph * PCOL
            for m in range(2):
                pso = sp.tile([128, 512], F32, tag="st", name="st")[:, 0:PCOL]
                for c in range(2):
                    nc.tensor.matmul(
                        pso[:], w_sb["o"][:, c, m * 128:(m + 1) * 128],
                        ctxT_sb[:, c, col0:col0 + PCOL],
                        start=(c == 0), stop=(c == 1))
                ot = ostage.tile([128, PCOL], F32, tag="ot", name="ot")
                bal.bias_add(
                    ot[:], pso[:],
                    b_sb["o"][:, m:m + 1].broadcast_to([128, PCOL]),
                    b_sb["o"][:, m:m + 1])
                nc.gpsimd.dma_start(
                    out=outT[m * 128:(m + 1) * 128, col0:col0 + PCOL],
                    in_=ot[:])


_PROGRAMS = {}
_CUR_NSLOT = None


def _get_program(nslot=None):
    global _CUR_NSLOT
    if nslot is None:
        nslot = _CUR_NSLOT if _CUR_NSLOT is not None else 96
    if nslot not in _PROGRAMS:
        _PROGRAMS[nslot] = _build_program(nslot)
    _CUR_NSLOT = nslot
    return _PROGRAMS[nslot]


def _slot_runs(wins):
    """Contiguous (window, start, end) runs of a window-sorted array,
    split into <=CAP chunks."""
    runs = []
    i, n = 0, len(wins)
    while i < n:
        w = wins[i]
        j = i
        while j < n and wins[j] == w:
            j += 1
        for s in range(i, j, CAP):
            runs.append((w, s, min(s + CAP, j)))
        i = j
    return runs


def _pack_core(x_b, xp_b, qidx, wins, nslot):
    nq = nslot * CAP
    runs = _slot_runs(wins)
    assert len(runs) <= nslot, f"slot overflow: {len(runs)} > {nslot}"
    slot_win = [r[0] for r in runs]
    slot_q = [qidx[r[1]:r[2]] for r in runs]
    while len(slot_win) < nslot:
        slot_win.append(slot_win[0])
        slot_q.append(np.empty([0], np.int64))

    owner = np.full([nq], -1, np.int64)
    xq = np.zeros([nq, C], np.float32)
    for si, qs in enumerate(slot_q):
        if len(qs):
            xq[si * CAP: si * CAP + len(qs)] = x_b[qs]
            owner[si * CAP: si * CAP + len(qs)] = qs
    xqT = np.ascontiguousarray(xq.T).astype(ml_dtypes.bfloat16)
    xpT = np.ascontiguousarray(
        xp_b[np.asarray(slot_win)].reshape(nslot * K, C).T
    ).astype(ml_dtypes.bfloat16)
    return xqT, xpT, owner


def make_in_maps(x, x_permute, idx_win, Wq, bq, Wk, bk, Wv, bv, Wo, bo):
    x = np.asarray(x, np.float32)
    xp = np.asarray(x_permute, np.float32)
    idx = np.asarray(idx_win)
    Wq = np.asarray(Wq, np.float32)
    Wo = np.asarray(Wo, np.float32)
    bo_f = (np.asarray(bo, np.float32) + Wo @ np.asarray(bv, np.float32))
    shared = {
        "wqT": np.ascontiguousarray(Wq.T * SCALE).astype(ml_dtypes.bfloat16),
        "wkT": np.ascontiguousarray(np.asarray(Wk, np.float32).T).astype(ml_dtypes.bfloat16),
        "wvT": np.ascontiguousarray(np.asarray(Wv, np.float32).T).astype(ml_dtypes.bfloat16),
        "woT": np.ascontiguousarray(Wo.T).astype(ml_dtypes.bfloat16),
        "bq": (np.asarray(bq, np.float32) * SCALE).astype(np.float32),
        "bo": bo_f.astype(np.float32),
    }
    # choose NSLOT from the worst core's real slot count, ceil to mult of 32
    cores = []
    max_slots = 0
    for core in range(NCORES):
        b, qtr = divmod(core, QTRS)
        order = np.argsort(idx[b], kind="stable")
        qidx = order[qtr * NLOC:(qtr + 1) * NLOC]
        wins = idx[b][qidx]
        cores.append((b, qidx, wins))
        max_slots = max(max_slots, len(_slot_runs(wins)))
    nslot = max(32, -(-max_slots // 32) * 32)

    in_maps, owners = [], []
    for b, qidx, wins in cores:
        xqT, xpT, owner = _pack_core(x[b], xp[b], qidx, wins, nslot)
        in_maps.append({"xqT": xqT, "xpT": xpT, **shared})
        owners.append((b, owner))
    global _CUR_NSLOT
    _CUR_NSLOT = nslot
    return in_maps, owners


def kernel(x, x_permute, idx_win, Wq, bq, Wk, bk, Wv, bv, Wo, bo):
    from concourse.bass_utils import run_bass_kernel_spmd

    in_maps, owners = make_in_maps(
        x, x_permute, idx_win, Wq, bq, Wk, bk, Wv, bv, Wo, bo)
    nc = _get_program()
    res = run_bass_kernel_spmd(nc, in_maps, list(range(NCORES)))
    out = np.zeros([B, N, C], np.float32)
    for core in range(NCORES):
        b, owner = owners[core]
        oT = np.asarray(res.results[core]["outT"], np.float32)
        valid = owner >= 0
        out[b][owner[valid]] = oT.T[valid]
    return out


# revision 46
# speedup vs baseline: 1.0112x; 1.0112x over previous
"""ClusterISAAttention Trainium2 kernel (8 NeuronCores, SPMD).

Host: per batch (2), stable-sort queries by window id, split into 4
contiguous quarters of 2048 queries -> 8 (batch, quarter) shards, one
per core.  Queries are grouped into "slots": one window per slot, <=32
queries per slot; slot count padded to NSLOT (multiple of 32, chosen
from the actual inputs, typically 96) so the SPMD program is static.
Padded queries/slots are dummies, discarded on unpack.

Bias folding (host): bk is dropped entirely (adds a per-query constant
to every logit in a row - softmax invariant); bv is folded into bo
(softmax rows sum to 1, so attn@(V+bv) = attn@V + bv and
bo' = bo + Wo @ bv); SCALE is folded into Wq/bq.  In-kernel k/v
evictions are therefore pure casts, distributed over the scalar,
vector and gpsimd engines by a greedy load balancer.

Device (per core), bf16 matmuls with fp32 psum:
  qT   = (Wq' @ xqT + bq') -> [256, NQ] dims-major
  kT   -> evicted into block-diag per-slot tables kbd[g][128, slot, 256]
          (4 heads per group; block-diag so one matmul covers 4 heads)
  vT   -> [128, slot, 128] tables (key half at 64*(h%2), rest zero),
          xbar-transposed per head-pair into block-diag
          vbd[pair][128(2x64 keys), slot, 64(2x32 dims)] - the zero half
          of vT lands as vbd's off-diagonal zeros
  S    = qT_slot.T @ kbd -> psum [32q x 4slots, 512] (query-major)
  A    = exp(S) (logits are O(1): no max subtraction), normalized by
         1/rowsum on DVE, key pads zeroed
  AT   = xbar transpose of A -> atbd[128(2x64 keys), pair, tile, 32q]
  ctxT = vbd.T @ atbd -> psum [128, 64] per slot -> ctxT2 bf16 [128, 2, NQ]
  outT = Wo @ ctxT + bo' -> fp32 [256, NQ] -> DRAM
Host: outT columns scattered back to original query order.

All xbar transposes go through nc.sync ONLY (concurrent transposes on
two HWDGE rings corrupt each other - HW-verified), and the xbar maps
src element index i -> dst partition i%128, dst mid i//128 (verified).
"""

import os
import sys
import numpy as np
import ml_dtypes

for _p in ("/opt/trn_rl_repo", "/root/.axon_site/_ro/trn_rl_repo"):
    if os.path.isdir(_p) and _p not in sys.path:
        sys.path.append(_p)

import concourse.bass as bass
import concourse.tile as tile
from concourse import bacc
from concourse import mybir

F32 = mybir.dt.float32
BF16 = mybir.dt.bfloat16
AF = mybir.ActivationFunctionType
ALU = mybir.AluOpType
AX = mybir.AxisListType

B, N, C, H, HD, W, K = 2, 8192, 256, 8, 32, 361, 49
SCALE = float(HD) ** -0.5

NCORES = 8
QTRS = 4
NLOC = N // QTRS         # 2048 queries per core
CAP = 32                 # queries per slot
SPP = 16                 # slots per phase (phase count = NSLOT/SPP)
KP = 64                  # padded keys per head block


class _EvictionBalancer:
    """Greedy distribution of psum-eviction ops over scalar/vector/gpsimd.

    Cost model (ns): fixed per-instruction overhead + free-elems/freq.
    """

    def __init__(self, nc):
        self.nc = nc
        self.load = {"scalar": 0.0, "vector": 0.0, "gpsimd": 0.0}

    def _cost(self, eng, elems):
        # calibrated against HW traces: vector CAST (bf16 out) ~399ns for
        # 294 elems, scalar ACTIVATE ~487, gpsimd TT ~1040 for 512
        if eng == "scalar":
            return 220.0 + elems / 1.2
        if eng == "vector":
            return 170.0 + elems / 1.05
        return 300.0 + elems / 0.75  # gpsimd (Q7 launch, slow impl)

    def charge(self, eng, elems):
        self.load[eng] += self._cost(eng, elems)

    def pick(self, elems, allowed=("scalar", "vector", "gpsimd")):
        eng = min(allowed, key=lambda e: self.load[e] + self._cost(e, elems))
        self.charge(eng, elems)
        return eng

    def copy(self, out, in_, allowed=("scalar", "vector")):
        # NOTE: gpsimd cannot access PSUM; psum evictions are scalar/vector
        eng = self.pick(out.free_size(), allowed)
        if eng == "scalar":
            self.nc.scalar.copy(out, in_)
        elif eng == "vector":
            self.nc.vector.tensor_copy(out, in_)
        else:
            self.nc.gpsimd.tensor_copy(out, in_)

    def bias_add(self, out, in_, bias_col, b_sb_scalar,
                 allowed=("scalar", "vector")):
        """out = in_ + bias (bias per-partition).  scalar uses activation,
        vector/gpsimd use tensor_tensor with a broadcast bias."""
        eng = self.pick(out.free_size(), allowed)
        if eng == "scalar":
            self.nc.scalar.activation(out, in_, AF.Identity, bias=b_sb_scalar)
        else:
            e = self.nc.vector if eng == "vector" else self.nc.gpsimd
            e.tensor_tensor(out=out, in0=in_, in1=bias_col, op=ALU.add)


def _build_program(nslot):
    assert nslot % SPP == 0 and nslot % 32 == 0 and nslot >= 32
    nq = nslot * CAP
    nc = bacc.Bacc("TRN2", target_bir_lowering=False, debug=False,
                   num_devices=NCORES)

    xqT = nc.declare_dram_parameter("xqT", [C, nq], BF16, isOutput=False).ap()
    xpT = nc.declare_dram_parameter("xpT", [C, nslot * K], BF16, isOutput=False).ap()
    wts = {
        nm: nc.declare_dram_parameter(f"w{nm}T", [C, C], BF16, isOutput=False).ap()
        for nm in ("q", "k", "v", "o")
    }
    bss = {
        nm: nc.declare_dram_parameter(f"b{nm}", [C], F32, isOutput=False).ap()
        for nm in ("q", "o")
    }
    outT = nc.declare_dram_parameter("outT", [C, nq], F32, isOutput=True).ap()

    with tile.TileContext(nc) as tc:
        _kernel_body(tc, xqT, xpT, wts, bss, outT, nslot)
    nc.compile()
    return nc


def _kernel_body(tc, xqT, xpT, wts, bss, outT, NSLOT):
    from contextlib import ExitStack

    PH = NSLOT // SPP        # phases (double-buffered)
    TPP = SPP // 4           # 4-slot tiles per phase
    SPC = SPP // 2           # slots per kv-proj chunk
    NQ = NSLOT * CAP

    nc = tc.nc
    bal = _EvictionBalancer(nc)
    ctx = ExitStack()
    with ctx:
        singles = ctx.enter_context(tc.tile_pool(name="singles", bufs=1))
        phpool = ctx.enter_context(tc.tile_pool(name="phase", bufs=2))
        pp = ctx.enter_context(tc.tile_pool(name="proj_ps", bufs=2, space="PSUM"))
        sp = ctx.enter_context(tc.tile_pool(name="s_ps", bufs=2, space="PSUM"))
        cp = ctx.enter_context(tc.tile_pool(name="ctx_ps", bufs=4, space="PSUM"))
        ostage = ctx.enter_context(tc.tile_pool(name="ostage", bufs=2))

        # ---- persistent SBUF ----
        w_sb = {nm: singles.tile([128, 2, C], BF16, tag=f"w_{nm}", name=f"w_{nm}") for nm in wts}
        b_sb = {nm: singles.tile([128, 2], F32, tag=f"b_{nm}", name=f"b_{nm}") for nm in bss}
        xq_sb = [singles.tile([128, NQ], BF16, tag=f"xq{c}", name=f"xq{c}") for c in range(2)]
        xp_sb = [singles.tile([128, NSLOT * K], BF16, tag=f"xp{c}", name=f"xp{c}") for c in range(2)]
        qT_sb = [singles.tile([128, NQ], BF16, tag=f"qT{g}", name=f"qT{g}") for g in range(2)]
        ctxT_sb = singles.tile([128, 2, NQ], BF16, tag="ctxT", name="ctxT")

        # ---- phase tiles: both parities created up front so their
        # one-time gap memsets can run at t=0 on idle engines (inside the
        # loop they'd queue behind load-blocked evictions) ----
        ptiles = []
        for par in range(2):
            ptiles.append(dict(
                kbd=[phpool.tile([128, SPP, 4 * KP], BF16, tag=f"kbd{g}", name=f"kbd{g}") for g in range(2)],
                vT=[phpool.tile([128, SPP, 2 * KP], BF16, tag=f"vT{m}", name=f"vT{m}") for m in range(2)],
                vbdf=[phpool.tile([128, SPP, 128], BF16, tag=f"vbdf{m}", name=f"vbdf{m}") for m in range(2)],
                a=phpool.tile([128, TPP, 4, 2 * KP], BF16, tag="a_sb", name="a_sb"),
                atbd=phpool.tile([128, TPP, 4, 4, CAP], BF16, tag="atbd", name="atbd"),
                den=phpool.tile([128, TPP, 8], F32, tag="den", name="den"),
                rec=phpool.tile([128, TPP, 8], F32, tag="rec", name="rec"),
            ))
        for par in range(2):
            T = ptiles[par]
            nc.vector.memset(T["kbd"][0][:], 0.0)
            nc.gpsimd.memset(T["kbd"][1][:], 0.0)
            nc.vector.memset(T["vT"][0][:], 0.0)
            nc.gpsimd.memset(T["vT"][1][:], 0.0)
            for a in range(2):
                (nc.vector if a == 0 else nc.gpsimd).memset(
                    T["a"][:, :, :, KP * a + K:KP * a + KP], 0.0)

        # ---- load inputs.  q-proj dependencies ride the otherwise-idle
        # sync (HWDGE) queue - its first transpose comes much later, so no
        # concurrency with transposes on the ring.  kv-side inputs go to
        # the gpsimd SWDGE queue, xp in per-phase chunks. ----
        nc.sync.dma_start(
            out=w_sb["q"][:], in_=wts["q"].rearrange("(s p) m -> p s m", p=128))
        nc.sync.dma_start(
            out=b_sb["q"][:], in_=bss["q"].rearrange("(s p) -> p s", p=128))
        # small first chunks so the first q-proj matmuls start ASAP
        xq_edges = [0, 512, 1024, NQ]
        for ck in range(len(xq_edges) - 1):
            lo, hi = xq_edges[ck], xq_edges[ck + 1]
            for c in range(2):
                nc.sync.dma_start(
                    out=xq_sb[c][:, lo:hi],
                    in_=xqT[c * 128:(c + 1) * 128, lo:hi])
        for nm in ("k", "v"):
            nc.gpsimd.dma_start(
                out=w_sb[nm][:], in_=wts[nm].rearrange("(s p) m -> p s m", p=128))
        for ph in range(PH):
            lo, hi = ph * SPP * K, (ph + 1) * SPP * K
            for c in range(2):
                nc.gpsimd.dma_start(out=xp_sb[c][:, lo:hi],
                                    in_=xpT[c * 128:(c + 1) * 128, lo:hi])
            if ph == 0:
                nc.gpsimd.dma_start(
                    out=w_sb["o"][:], in_=wts["o"].rearrange("(s p) m -> p s m", p=128))
                nc.gpsimd.dma_start(
                    out=b_sb["o"][:], in_=bss["o"].rearrange("(s p) -> p s", p=128))

        # ---- q projection: chunk nch covers exactly phase nch's queries
        # (SPP*CAP = 512).  Chunks 0-1 run up front; chunk p+2 is deferred
        # into phase p's body to shorten the serial head. ----
        def do_qproj(nch):
            for m in range(2):
                ps = pp.tile([128, 512], F32, tag="ps", name="ps")
                for c in range(2):
                    nc.tensor.matmul(
                        ps[:], w_sb["q"][:, c, m * 128:(m + 1) * 128],
                        xq_sb[c][:, nch * 512:(nch + 1) * 512],
                        start=(c == 0), stop=(c == 1))
                bal.bias_add(
                    qT_sb[m][:, nch * 512:(nch + 1) * 512], ps[:],
                    b_sb["q"][:, m:m + 1].broadcast_to([128, 512]),
                    b_sb["q"][:, m:m + 1])

        for ph in range(PH):
            if ph > 0 and ph + 2 < NQ // 512:
                do_qproj(ph + 2)
            T = ptiles[ph % 2]
            kbd_sb, vT_sb, vbdf_sb = T["kbd"], T["vT"], T["vbdf"]
            a_sb, atbd_sb, den_sb, rec_sb = T["a"], T["atbd"], T["den"], T["rec"]

            # k/v projections for this phase's SPP slots (evictions are
            # pure casts - biases folded on host - balanced over engines)
            for proj in ("k", "v"):
                for ch in range(2):
                    col0 = ph * SPP * K + ch * SPC * K
                    for m in range(2):
                        ps = pp.tile([128, 512], F32, tag="ps", name="ps")
                        for c in range(2):
                            nc.tensor.matmul(
                                ps[:, 0:SPC * K], w_sb[proj][:, c, m * 128:(m + 1) * 128],
                                xp_sb[c][:, col0:col0 + SPC * K],
                                start=(c == 0), stop=(c == 1))
                        if proj == "k":
                            for bb in range(4):
                                bal.copy(
                                    kbd_sb[m][32 * bb:32 * bb + 32,
                                              ch * SPC:(ch + 1) * SPC,
                                              KP * bb:KP * bb + K],
                                    ps[32 * bb:32 * bb + 32, 0:SPC * K].rearrange(
                                        "p (s k) -> p s k", k=K))
                        else:
                            for hh in range(4):
                                ko = KP * (hh % 2)
                                bal.copy(
                                    vT_sb[m][32 * hh:32 * hh + 32,
                                             ch * SPC:(ch + 1) * SPC, ko:ko + K],
                                    ps[32 * hh:32 * hh + 32, 0:SPC * K].rearrange(
                                        "p (s k) -> p s k", k=K))

            # vT -> block-diag vbd: per-chunk xbar transposes so each
            # fires as soon as its half of the evictions lands
            for chh in range(2):
                sl = slice(chh * SPC, (chh + 1) * SPC)
                for m in range(2):
                    nc.sync.dma_start_transpose(
                        out=vbdf_sb[m][:, sl, :], in_=vT_sb[m][:, sl, :])

            if ph == 0:
                for nch in range(min(3, NQ // 512)):
                    do_qproj(nch)

            # logits + exp + rowsum per 4-slot tile
            for t in range(TPP):
                st = sp.tile([128, 512], F32, tag="st", name="st")
                for sl in range(4):
                    s_ph = t * 4 + sl
                    qcol = (ph * SPP + s_ph) * CAP
                    for g in range(2):
                        nc.tensor.matmul(
                            st[32 * sl:32 * sl + 32, 256 * g:256 * g + 256],
                            qT_sb[g][:, qcol:qcol + CAP],
                            kbd_sb[g][:, s_ph, :],
                            start=(g == 0), stop=(g == 1),
                            skip_group_check=True, tile_position=(0, 32 * sl))
                # exp / rowsum / normalize only touch the 49 valid key
                # columns; the 64-pads are zeroed once below and stay zero
                nc.scalar.activation(
                    a_sb[:, t, :, :].rearrange("p x (a j) -> p x a j", a=2)[:, :, :, 0:K],
                    st[:].rearrange("p (x a j) -> p x a j", x=4, a=2)[:, :, :, 0:K],
                    AF.Exp)
                bal.charge("scalar", 560)
                bal.charge("vector", 550)
                nc.vector.tensor_reduce(
                    out=den_sb[:, t, :].rearrange("p (x a) -> p x a", x=4),
                    in_=a_sb[:, t, :, :].rearrange("p x (a j) -> p x a j", a=2)[:, :, :, 0:K],
                    axis=AX.X, op=ALU.add)
                # per-tile reciprocal: shortens the reduce->normalize chain
                nc.vector.reciprocal(rec_sb[:, t, :], den_sb[:, t, :])
                bal.charge("vector", 200)
                a4 = a_sb[:, t, :, :].rearrange("p x (a j) -> p x a j", a=2)[:, :, :, 0:K]
                r4 = rec_sb[:, t, :].rearrange("p (x a) -> p x a", x=4) \
                    .unsqueeze(3).broadcast_to([128, 4, 2, K])
                eng = bal.pick(392, ("vector", "gpsimd"))
                e = nc.vector if eng == "vector" else nc.gpsimd
                e.tensor_tensor(out=a4, in0=a4, in1=r4, op=ALU.mult)
                if t % 2 == 1:
                    t2 = t - 1
                    nc.sync.dma_start_transpose(
                        out=atbd_sb[:, t2:t2 + 2, :, :, :].rearrange(
                            "p t x c q -> p (t x) (c q)"),
                        in_=a_sb[:, t2:t2 + 2, :, :])
            # A -> atbd: two half-phase xbar transposes (t-major layout
            # keeps each slice a contiguous 2D AP; dst inner = src
            # partitions = (cq, q)).  NOTE: issued inside the t-loop via
            # the marker below.

            # ctx: 4 pair-matmuls per slot; 2 slots share one psum tile so
            # the ctxT eviction runs half as often at double width
            for s2 in range(SPP // 2):
                cps = cp.tile([128, 2, 2 * CAP], F32, tag="cps", name="cps")
                for si in range(2):
                    s_ph = 2 * s2 + si
                    t, cq = divmod(s_ph, 4)
                    for p in range(4):
                        m, q = divmod(p, 2)
                        nc.tensor.matmul(
                            cps[64 * (p % 2):64 * (p % 2) + 64, si,
                                CAP * (p // 2):CAP * (p // 2) + CAP],
                            vbdf_sb[m][:, s_ph, 64 * q:64 * q + 64],
                            atbd_sb[:, t, p, cq, :],
                            start=(p < 2), stop=(p >= 2),
                            skip_group_check=True, tile_position=(0, 64 * (p % 2)))
                col = (ph * SPP + 2 * s2) * CAP
                cview = cps[:].rearrange("p s (j q) -> p j s q", j=2)
                bal.copy(
                    ctxT_sb[:, :, col:col + 2 * CAP].rearrange(
                        "p j (s q) -> p j s q", s=2), cview)

            # ---- output projection for this phase's columns (psum reuses
            # the S-tile pool slots: no extra banks) ----
            PCOL = SPP * CAP
            col0 = ph * PCOL
            for m in range(2):
                pso = sp.tile([128, 512], F32, tag="st", name="st")[:, 0:PCOL]
                for c in range(2):
                    nc.tensor.matmul(
                        pso[:], w_sb["o"][:, c, m * 128:(m + 1) * 128],
                        ctxT_sb[:, c, col0:col0 + PCOL],
                        start=(c == 0), stop=(c == 1))
                ot = ostage.tile([128, PCOL], F32, tag="ot", name="ot")
                bal.bias_add(
                    ot[:], pso[:],
                    b_sb["o"][:, m:m + 1].broadcast_to([128, PCOL]),
                    b_sb["o"][:, m:m + 1])
                nc.gpsimd.dma_start(
                    out=outT[m * 128:(m + 1) * 128, col0:col0 + PCOL],
                    in_=ot[:])


_PROGRAMS = {}
_CUR_NSLOT = None


def _get_program(nslot=None):
    global _CUR_NSLOT
    if nslot is None:
        nslot = _CUR_NSLOT if _CUR_NSLOT is not None else 96
    if nslot not in _PROGRAMS:
        _PROGRAMS[nslot] = _build_program(nslot)
    _CUR_NSLOT = nslot
    return _PROGRAMS[nslot]


def _slot_runs(wins):
    """Contiguous (window, start, end) runs of a window-sorted array,
    split into <=CAP chunks."""
    runs = []
    i, n = 0, len(wins)
    while i < n:
        w = wins[i]
        j = i
        while j < n and wins[j] == w:
            j += 1
        for s in range(i, j, CAP):
            runs.append((w, s, min(s + CAP, j)))
        i = j
    return runs


def _pack_core(x_b, xp_b, qidx, wins, nslot):
    nq = nslot * CAP
    runs = _slot_runs(wins)
    assert len(runs) <= nslot, f"slot overflow: {len(runs)} > {nslot}"
    slot_win = [r[0] for r in runs]
    slot_q = [qidx[r[1]:r[2]] for r in runs]
    while len(slot_win) < nslot:
        slot_win.append(slot_win[0])
        slot_q.append(np.empty([0], np.int64))

    owner = np.full([nq], -1, np.int64)
    xq = np.zeros([nq, C], np.float32)
    for si, qs in enumerate(slot_q):
        if len(qs):
            xq[si * CAP: si * CAP + len(qs)] = x_b[qs]
            owner[si * CAP: si * CAP + len(qs)] = qs
    xqT = np.ascontiguousarray(xq.T).astype(ml_dtypes.bfloat16)
    xpT = np.ascontiguousarray(
        xp_b[np.asarray(slot_win)].reshape(nslot * K, C).T
    ).astype(ml_dtypes.bfloat16)
    return xqT, xpT, owner


def make_in_maps(x, x_permute, idx_win, Wq, bq, Wk, bk, Wv, bv, Wo, bo):
    x = np.asarray(x, np.float32)
    xp = np.asarray(x_permute, np.float32)
    idx = np.asarray(idx_win)
    Wq = np.asarray(Wq, np.float32)
    Wo = np.asarray(Wo, np.float32)
    bo_f = (np.asarray(bo, np.float32) + Wo @ np.asarray(bv, np.float32))
    shared = {
        "wqT": np.ascontiguousarray(Wq.T * SCALE).astype(ml_dtypes.bfloat16),
        "wkT": np.ascontiguousarray(np.asarray(Wk, np.float32).T).astype(ml_dtypes.bfloat16),
        "wvT": np.ascontiguousarray(np.asarray(Wv, np.float32).T).astype(ml_dtypes.bfloat16),
        "woT": np.ascontiguousarray(Wo.T).astype(ml_dtypes.bfloat16),
        "bq": (np.asarray(bq, np.float32) * SCALE).astype(np.float32),
        "bo": bo_f.astype(np.float32),
    }
    # choose NSLOT from the worst core's real slot count, ceil to mult of 32
    cores = []
    max_slots = 0
    for core in range(NCORES):
        b, qtr = divmod(core, QTRS)
        order = np.argsort(idx[b], kind="stable")
        qidx = order[qtr * NLOC:(qtr + 1) * NLOC]
        wins = idx[b][qidx]
        cores.append((b, qidx, wins))
        max_slots = max(max_slots, len(_slot_runs(wins)))
    nslot = max(32, -(-max_slots // 32) * 32)

    in_maps, owners = [], []
    for b, qidx, wins in cores:
        xqT, xpT, owner = _pack_core(x[b], xp[b], qidx, wins, nslot)
        in_maps.append({"xqT": xqT, "xpT": xpT, **shared})
        owners.append((b, owner))
    global _CUR_NSLOT
    _CUR_NSLOT = nslot
    return in_maps, owners


def kernel(x, x_permute, idx_win, Wq, bq, Wk, bk, Wv, bv, Wo, bo):
    from concourse.bass_utils import run_bass_kernel_spmd

    in_maps, owners = make_in_maps(
        x, x_permute, idx_win, Wq, bq, Wk, bk, Wv, bv, Wo, bo)
    nc = _get_program()
    res = run_bass_kernel_spmd(nc, in_maps, list(range(NCORES)))
    out = np.zeros([B, N, C], np.float32)
    for core in range(NCORES):
        b, owner = owners[core]
        oT = np.asarray(res.results[core]["outT"], np.float32)
        valid = owner >= 0
        out[b][owner[valid]] = oT.T[valid]
    return out
